# revision 1
# baseline (speedup 1.0000x reference)
"""DDI regularizer loss kernel for 8 Trainium2 NeuronCores.

reference semantics:
    b = (ddi > 0); S = max(b, b.T) with zero diagonal; U = triu(S, k=1)
    normalizer = max(U.sum(), 1.0)
    xu = drug_probs @ U; penalties = sum(xu * drug_probs, axis=1) / normalizer
    return penalties.mean()

Identity used here:
    mean_i(x_i^T U x_i) = <U, X^T X> / B
so the kernel computes G = X^T X only on upper-triangular 128x512 tiles
(contraction over the batch is the natural PE layout - no transposes of X),
masks each G tile with U's tile (built on device from bf16 ddi slices) and
reduces.  40 real tiles + 8 dummy slots are distributed 6-per-core across the
8 cores; each core returns per-partition partial sums of (U*G) and of U, and
the host combines 8 tiny vectors into the final scalar.

The matmuls run in fp8 e5m2 with DoubleRow packing (two 128-row batch chunks
per matmul, fp32 PSUM accumulation).  For this problem's uniform[0,1) inputs
the e5m2 quantization error on the final scalar is ~4e-6 relative (bf16:
~1e-6, validated against the fp32 reference); set MODE="bf16" to fall back.
"""

import sys

for _p in ("/opt/trn_rl_repo", "/root/.axon_site/_ro/trn_rl_repo"):
    if _p not in sys.path:
        sys.path.insert(0, _p)

import numpy as np
import ml_dtypes

B, D = 4096, 2048
NBLK = 128  # lhs row-block width
NCOL = 512  # rhs col-block width
NSLOT = 6  # tile slots per core
MODE = "fp8dr"  # "fp8dr" | "bf16"

# (J, [row-block indices; -1 = dummy slot]) per core.  Tile (i, J) covers
# G[128i:128i+128, 512J:512J+512]; it exists iff i <= 4J+3 (touches the
# strict upper triangle).
CORE_ASSIGN = [
    (3, [0, 1, 2, 3, 4, 5]),
    (3, [6, 7, 8, 9, 10, 11]),
    (3, [12, 13, 14, 15, -1, -1]),
    (2, [0, 1, 2, 3, 4, 5]),
    (2, [6, 7, 8, 9, 10, 11]),
    (1, [0, 1, 2, 3, 4, 5]),
    (1, [6, 7, -1, -1, -1, -1]),
    (0, [0, 1, 2, 3, -1, -1]),
]

NIN = NCOL + NBLK * NSLOT  # 1280 columns in the merged X input

_CACHE = {}


def _x_dtype():
    return ml_dtypes.float8_e5m2 if MODE == "fp8dr" else ml_dtypes.bfloat16


def _build():
    import concourse.bass as bass
    import concourse.mybir as mybir
    from concourse import bacc
    from concourse.tile import TileContext

    f32 = mybir.dt.float32
    bf16 = mybir.dt.bfloat16
    fp8 = mybir.dt.float8e5
    xdt = fp8 if MODE == "fp8dr" else bf16
    op = mybir.AluOpType

    nc = bacc.Bacc("TRN2", target_bir_lowering=False, debug=False, num_devices=8)

    # xin = [X columns for rhs | X columns for the 6 lhs slots], one DRAM read
    xin_d = nc.dram_tensor("xin", [B, NIN], xdt, kind="ExternalInput")
    ddiA_d = nc.dram_tensor("ddiA", [NBLK, NCOL * NSLOT], bf16, kind="ExternalInput")
    ddiB_d = nc.dram_tensor("ddiB", [NCOL, NBLK * NSLOT], bf16, kind="ExternalInput")
    thr_d = nc.dram_tensor("thr", [128, NSLOT], f32, kind="ExternalInput")
    out_d = nc.dram_tensor("out", [128, 2 * NSLOT], f32, kind="ExternalOutput")

    with TileContext(nc) as tc:
        with (
            tc.tile_pool(name="const", bufs=1) as cpool,
            tc.tile_pool(name="masks", bufs=NSLOT) as mpool,
            tc.tile_pool(name="io", bufs=6) as iopool,
            tc.tile_pool(name="psum", bufs=NSLOT, space="PSUM") as ppool,
            tc.tile_pool(name="scr", bufs=3) as spool,
        ):
            # --- constants from native iota (no DMA) ---
            iota = cpool.tile([128, NCOL], f32, tag="iota")
            nc.gpsimd.iota(
                iota,
                pattern=[[1, NCOL]],
                base=0,
                channel_multiplier=0,
                allow_small_or_imprecise_dtypes=True,
            )
            iotap = cpool.tile([128, 1], f32, tag="iotap")
            nc.gpsimd.iota(
                iotap,
                pattern=[[1, 1]],
                base=0,
                channel_multiplier=1,
                allow_small_or_imprecise_dtypes=True,
            )
            idn = cpool.tile([128, NBLK], bf16, tag="idn")
            nc.vector.tensor_scalar(
                out=idn, in0=iota[:, :NBLK], scalar1=iotap, scalar2=None,
                op0=op.is_equal,
            )

            # --- mirror-ddi blocks: load naturally (plain copies only - DMA
            # transpose would force xbar-mode serialization), binarize on DVE,
            # transpose the binary tiles on the otherwise-idle PE ---
            ddiB_sb = cpool.tile([128, 4, NBLK * NSLOT], bf16, tag="ddiB")
            nc.sync.dma_start(
                out=ddiB_sb, in_=ddiB_d.ap().rearrange("(s p) c -> p s c", p=128)
            )
            binB = cpool.tile([128, 4, NBLK * NSLOT], bf16, tag="binB")
            nc.vector.tensor_scalar(
                out=binB, in0=ddiB_sb, scalar1=0.0, scalar2=None, op0=op.is_gt
            )
            ddiBT = []
            with tc.tile_pool(name="tpp", bufs=2, space="PSUM") as tppool:
                for t in range(NSLOT):
                    bt = cpool.tile([NBLK, NCOL], bf16, tag=f"ddiBT{t}")
                    for s in range(4):
                        pst = tppool.tile([128, NBLK], bf16, tag="tp", name=f"tp{t}_{s}")
                        nc.tensor.transpose(
                            out=pst,
                            in_=binB[:, s, t * NBLK : (t + 1) * NBLK],
                            identity=idn,
                        )
                        nc.vector.tensor_copy(
                            out=bt[:, s * NBLK : (s + 1) * NBLK], in_=pst
                        )
                    ddiBT.append(bt)

            # --- G tiles: accumulating matmuls, k-outer so the X stream is
            # consumed strictly in order ---
            psums = [
                ppool.tile([128, NCOL], f32, tag="gps", name=f"gps{t}")
                for t in range(NSLOT)
            ]
            if MODE == "fp8dr":
                NK = B // 256  # two 128-row chunks per DoubleRow matmul
                xin_ap = xin_d.ap().rearrange("(k i p) c -> k p i c", i=2, p=128)
                for k in range(NK):
                    xt = iopool.tile([128, 2, NIN], xdt, tag="xt")
                    nc.sync.dma_start(out=xt, in_=xin_ap[k])
                    for t in range(NSLOT):
                        c0 = NCOL + t * NBLK
                        nc.tensor.matmul(
                            out=psums[t],
                            lhsT=xt[:, :, c0 : c0 + NBLK],
                            rhs=xt[:, :, 0:NCOL],
                            start=(k == 0),
                            stop=(k == NK - 1),
                            perf_mode=mybir.MatmulPerfMode.DoubleRow,
                        )
            else:
                NK = B // 128
                for k in range(NK):
                    xt = iopool.tile([128, NIN], xdt, tag="xt")
                    nc.sync.dma_start(
                        out=xt, in_=xin_d.ap()[128 * k : 128 * k + 128, :]
                    )
                    for t in range(NSLOT):
                        c0 = NCOL + t * NBLK
                        nc.tensor.matmul(
                            out=psums[t],
                            lhsT=xt[:, c0 : c0 + NBLK],
                            rhs=xt[:, 0:NCOL],
                            start=(k == 0),
                            stop=(k == NK - 1),
                        )

            # --- ddi/thr loads ride SWDGE mid-stream (never delay the X
            # stream, and stay behind the transposes in the global DMA order
            # so there is exactly one xbar-mode transition) ---
            thr_sb = cpool.tile([128, NSLOT], f32, tag="thr")
            nc.gpsimd.dma_start(out=thr_sb, in_=thr_d.ap())
            ddiA_sb = cpool.tile([NBLK, NCOL * NSLOT], bf16, tag="ddiA")
            for t in range(NSLOT):
                nc.gpsimd.dma_start(
                    out=ddiA_sb[:, t * NCOL : (t + 1) * NCOL],
                    in_=ddiA_d.ap()[:, t * NCOL : (t + 1) * NCOL],
                )

            # masks on DVE, overlapped with the matmul phase:
            # U_tile = max(A>0, B.T>0) * (col > row)
            out_sb = cpool.tile([128, 2 * NSLOT], f32, tag="out")
            masks = []
            for t in range(NSLOT):
                sel = spool.tile([128, NCOL], bf16, tag="sel")
                nc.vector.tensor_scalar(
                    out=sel, in0=iota, scalar1=thr_sb[:, t : t + 1],
                    scalar2=None, op0=op.is_gt,
                )
                binA = spool.tile([128, NCOL], bf16, tag="binA")
                nc.vector.tensor_scalar(
                    out=binA, in0=ddiA_sb[:, t * NCOL : (t + 1) * NCOL],
                    scalar1=0.0, scalar2=None, op0=op.is_gt,
                )
                mraw = spool.tile([128, NCOL], bf16, tag="mraw")
                nc.vector.tensor_tensor(out=mraw, in0=binA, in1=ddiBT[t], op=op.max)
                mask = mpool.tile([128, NCOL], bf16, tag="mask")
                nc.vector.tensor_tensor(out=mask, in0=mraw, in1=sel, op=op.mult)
                masks.append(mask)
                # normalizer partial: sum(mask) fused into one op
                mjunk = spool.tile([128, NCOL], bf16, tag="mjunk")
                nc.vector.tensor_scalar(
                    out=mjunk, in0=mask, scalar1=1.0, scalar2=None, op0=op.mult,
                    op1=op.add,  # reduce op for accum_out
                    accum_out=out_sb[:, NSLOT + t : NSLOT + t + 1],
                )

            # --- masked reduction: sum(G * mask), one fused op per slot ---
            for t in range(NSLOT):
                gjunk = spool.tile([128, NCOL], f32, tag="gjunk")
                nc.vector.scalar_tensor_tensor(
                    out=gjunk, in0=psums[t], scalar=1.0, in1=masks[t],
                    op0=op.mult, op1=op.mult,
                    accum_out=out_sb[:, t : t + 1],
                )

            nc.gpsimd.dma_start(out=out_d.ap(), in_=out_sb)

    nc.compile()
    return nc


def _in_maps(drug_probs, ddi_matrix):
    xdt = _x_dtype()
    xq = drug_probs.astype(xdt)
    db = ddi_matrix.astype(ml_dtypes.bfloat16)
    zero_x = np.zeros((B, NBLK), dtype=xdt)
    zero_a = np.zeros((NBLK, NCOL), dtype=ml_dtypes.bfloat16)
    zero_b = np.zeros((NCOL, NBLK), dtype=ml_dtypes.bfloat16)
    maps = []
    for J, slots in CORE_ASSIGN:
        xin = np.concatenate(
            [xq[:, J * NCOL : (J + 1) * NCOL]]
            + [xq[:, i * NBLK : (i + 1) * NBLK] if i >= 0 else zero_x for i in slots],
            axis=1,
        )
        ddiA = np.concatenate(
            [
                db[i * NBLK : (i + 1) * NBLK, J * NCOL : (J + 1) * NCOL]
                if i >= 0
                else zero_a
                for i in slots
            ],
            axis=1,
        )
        ddiB = np.concatenate(
            [
                db[J * NCOL : (J + 1) * NCOL, i * NBLK : (i + 1) * NBLK]
                if i >= 0
                else zero_b
                for i in slots
            ],
            axis=1,
        )
        p = np.arange(128, dtype=np.float32)[:, None]
        thr = np.concatenate(
            [
                p + np.float32(i * NBLK - J * NCOL)
                if i >= 0
                else np.full((128, 1), 1e9, np.float32)
                for i in slots
            ],
            axis=1,
        )
        maps.append(
            {
                "xin": np.ascontiguousarray(xin),
                "ddiA": np.ascontiguousarray(ddiA),
                "ddiB": np.ascontiguousarray(ddiB),
                "thr": np.ascontiguousarray(thr),
            }
        )
    return maps


def kernel(drug_probs, ddi_matrix, **_run_kwargs):
    from concourse.bass_utils import run_bass_kernel_spmd

    if "nc" not in _CACHE:
        _CACHE["nc"] = _build()
    nc = _CACHE["nc"]

    maps = _in_maps(np.asarray(drug_probs), np.asarray(ddi_matrix))
    res = run_bass_kernel_spmd(nc, maps, list(range(8)), **_run_kwargs)
    _CACHE["last_result"] = res

    gsum = 0.0
    msum = 0.0
    for core_out in res.results:
        o = core_out["out"].astype(np.float64)
        gsum += o[:, :NSLOT].sum()
        msum += o[:, NSLOT:].sum()
    normalizer = max(msum, 1.0)
    return np.asarray(gsum / (B * normalizer), dtype=np.float32)



# revision 3
# speedup vs baseline: 1.0395x; 1.0395x over previous
"""DDI regularizer loss kernel for 8 Trainium2 NeuronCores.

reference semantics:
    b = (ddi > 0); S = max(b, b.T) with zero diagonal; U = triu(S, k=1)
    normalizer = max(U.sum(), 1.0)
    xu = drug_probs @ U; penalties = sum(xu * drug_probs, axis=1) / normalizer
    return penalties.mean()

Identity:  mean_i(x_i^T U x_i) = <U, X^T X> / B

G = X^T X is computed on 40 upper-triangular 128x512 tiles, 6 slots per
core (SPMD: one program, per-core content via host-permuted inputs, with
dummy slots zero-filled and masked out).  fp8 e5m2 DoubleRow matmuls,
fp32 PSUM accumulation.

vs. the previous revision:
  - U (and the normalizer) are precomputed on the host - a pure function
    of ddi_matrix - and shipped as 0/1 bf16 strips.  No on-device
    binarize / PE transposes / iota / threshold compare.
  - X ships pre-transposed [128, 40960] so every DMA line is 5120B
    contiguous (>4KB saturates the bus), 8 group-DMAs on the SWDGE
    queue, issued BEFORE everything else; the mask DMA rides behind the
    fourth group.  (Previously 1.6MB of ddi loads sat in front of the X
    stream and delayed the first matmul by ~4.5us.)
  - A short burst of dummy matmuls at program start ramps the PE
    p-state off the 0.65/1.2GHz launch clocks while the first X group
    is still in flight.
"""

import sys

for _p in ("/opt/trn_rl_repo", "/root/.axon_site/_ro/trn_rl_repo"):
    if _p not in sys.path:
        sys.path.insert(0, _p)

import numpy as np
import ml_dtypes

B, D = 4096, 2048
NBLK = 128
NCOL = 512
NSLOT = 6
NIN = NCOL + NBLK * NSLOT  # 1280 cols per core
NCHUNK = 16  # 256-row DoubleRow chunks
NGRP = 8  # 2 chunks per DMA group
GBYTES = 2 * 2 * NIN  # 5120 per-partition bytes per group
N_WARM = 7

# (J, [row-block indices; -1 = dummy]) per core; tile (i,J) covers
# G[128i:128(i+1), 512J:512(J+1)], exists iff i <= 4J+3.
CORE_ASSIGN = [
    (3, [0, 1, 2, 3, 4, 5]),
    (3, [6, 7, 8, 9, 10, 11]),
    (3, [12, 13, 14, 15, -1, -1]),
    (2, [0, 1, 2, 3, 4, 5]),
    (2, [6, 7, 8, 9, 10, 11]),
    (1, [0, 1, 2, 3, 4, 5]),
    (1, [6, 7, -1, -1, -1, -1]),
    (0, [0, 1, 2, 3, -1, -1]),
]

_CACHE = {}


def _build():
    import concourse.mybir as mybir
    from concourse import bacc
    from concourse.tile import TileContext

    f32 = mybir.dt.float32
    bf16 = mybir.dt.bfloat16
    fp8 = mybir.dt.float8e5
    op = mybir.AluOpType

    nc = bacc.Bacc("TRN2", target_bir_lowering=False, debug=False, num_devices=8)

    # [128, 40960] fp8: partition p, chunk k, half i holds the 1280 xin
    # columns of batch row 256k + 128i + p at byte offset 2560k + 1280i.
    xin_d = nc.dram_tensor("xin", [128, NCHUNK * 2 * NIN], fp8, kind="ExternalInput")
    msk_d = nc.dram_tensor("msk", [128, NSLOT * NCOL], bf16, kind="ExternalInput")
    out_d = nc.dram_tensor("out", [128, NSLOT], f32, kind="ExternalOutput")

    with TileContext(nc) as tc:
        with (
            tc.tile_pool(name="const", bufs=1) as cpool,
            tc.tile_pool(name="xg", bufs=NGRP) as xpool,
            tc.tile_pool(name="psum", bufs=NSLOT, space="PSUM") as ppool,
            tc.tile_pool(name="scr", bufs=2) as spool,
        ):
            psums = [
                ppool.tile([128, NCOL], f32, tag="gps", name=f"gps{t}")
                for t in range(NSLOT)
            ]

            # --- X stream first, masks behind group 3 ---
            grps = []
            msk = None
            for g in range(NGRP):
                xt = xpool.tile([128, 2, 2, NIN], fp8, tag="xg", name=f"xg{g}")
                nc.gpsimd.dma_start(
                    out=xt,
                    in_=xin_d.ap()[:, g * GBYTES : (g + 1) * GBYTES].rearrange(
                        "p (k i c) -> p k i c", k=2, i=2
                    ),
                )
                grps.append(xt)
                if g == 3:
                    msk = cpool.tile([128, NSLOT * NCOL], bf16, tag="msk")
                    nc.gpsimd.dma_start(out=msk, in_=msk_d.ap())

            # --- PE p-state warmup on a zeroed tile; result overwritten by
            # the k=0 start=True matmul into the same PSUM bank ---
            warm = cpool.tile([128, 2, NCOL], fp8, tag="warm")
            nc.gpsimd.memset(warm, 0)
            for _ in range(N_WARM):
                nc.tensor.matmul(
                    out=psums[0],
                    lhsT=warm[:, :, :NBLK],
                    rhs=warm,
                    start=True,
                    stop=True,
                    perf_mode=mybir.MatmulPerfMode.DoubleRow,
                )

            # --- Gram matmuls: 6 x N=512 fp8 DR per 256-row chunk ---
            for k in range(NCHUNK):
                xt = grps[k // 2]
                kk = k % 2
                for t in range(NSLOT):
                    c0 = NCOL + t * NBLK
                    nc.tensor.matmul(
                        out=psums[t],
                        lhsT=xt[:, kk, :, c0 : c0 + NBLK],
                        rhs=xt[:, kk, :, 0:NCOL],
                        start=(k == 0),
                        stop=(k == NCHUNK - 1),
                        perf_mode=mybir.MatmulPerfMode.DoubleRow,
                    )

            # --- fused mask-multiply-reduce, one op per slot ---
            out_sb = cpool.tile([128, NSLOT], f32, tag="out")
            for t in range(NSLOT):
                junk = spool.tile([128, NCOL], f32, tag="junk")
                nc.vector.scalar_tensor_tensor(
                    out=junk,
                    in0=psums[t],
                    scalar=1.0,
                    in1=msk[:, t * NCOL : (t + 1) * NCOL],
                    op0=op.mult,
                    op1=op.mult,
                    accum_out=out_sb[:, t : t + 1],
                )

            nc.gpsimd.dma_start(out=out_d.ap(), in_=out_sb)

    nc.compile()
    return nc


def _in_maps(drug_probs, ddi_matrix):
    xq = drug_probs.astype(ml_dtypes.float8_e5m2)
    bpos = ddi_matrix > 0
    U = np.triu(bpos | bpos.T, 1).astype(ml_dtypes.bfloat16)
    normalizer = max(float(np.count_nonzero(U)), 1.0)

    zero_x = np.zeros((B, NBLK), dtype=ml_dtypes.float8_e5m2)
    zero_m = np.zeros((NBLK, NCOL), dtype=ml_dtypes.bfloat16)
    maps = []
    for J, slots in CORE_ASSIGN:
        xin = np.concatenate(
            [xq[:, J * NCOL : (J + 1) * NCOL]]
            + [xq[:, i * NBLK : (i + 1) * NBLK] if i >= 0 else zero_x for i in slots],
            axis=1,
        )  # [4096, 1280]
        # pre-transpose: [128, 16*2*1280], 5120B contiguous per group line
        xin = np.ascontiguousarray(
            xin.reshape(NCHUNK, 2, 128, NIN)
            .transpose(2, 0, 1, 3)
            .reshape(128, NCHUNK * 2 * NIN)
        )
        msk = np.concatenate(
            [
                U[i * NBLK : (i + 1) * NBLK, J * NCOL : (J + 1) * NCOL]
                if i >= 0
                else zero_m
                for i in slots
            ],
            axis=1,
        )
        maps.append({"xin": xin, "msk": np.ascontiguousarray(msk)})
    return maps, normalizer


def kernel(drug_probs, ddi_matrix, **_run_kwargs):
    from concourse.bass_utils import run_bass_kernel_spmd

    if "nc" not in _CACHE:
        _CACHE["nc"] = _build()
    nc = _CACHE["nc"]

    maps, normalizer = _in_maps(np.asarray(drug_probs), np.asarray(ddi_matrix))
    res = run_bass_kernel_spmd(nc, maps, list(range(8)), **_run_kwargs)
    _CACHE["last_result"] = res

    gsum = 0.0
    for core_out in res.results:
        gsum += core_out["out"].astype(np.float64).sum()
    return np.asarray(gsum / (B * normalizer), dtype=np.float32)


# revision 7
# speedup vs baseline: 1.2041x; 1.1584x over previous
"""DDI regularizer loss kernel for 8 Trainium2 NeuronCores.

reference semantics:
    b = (ddi > 0); S = max(b, b.T) with zero diagonal; U = triu(S, k=1)
    normalizer = max(U.sum(), 1.0)
    xu = drug_probs @ U; penalties = sum(xu * drug_probs, axis=1) / normalizer
    return penalties.mean()

Identity:  mean_i(x_i^T U x_i) = <U, X^T X> / B

G = X^T X is computed on 40 upper-triangular 128x512 tiles, 6 slots per
core (SPMD: one program, per-core content via host-permuted inputs, with
dummy slots zero-filled and masked out).  fp8 e5m2 DoubleRow matmuls,
fp32 PSUM accumulation.

vs. the previous revision:
  - U (and the normalizer) are precomputed on the host - a pure function
    of ddi_matrix - and shipped as 0/1 bf16 strips.  No on-device
    binarize / PE transposes / iota / threshold compare.
  - X ships pre-transposed [128, 40960] so every DMA line is 5120B
    contiguous (>4KB saturates the bus), 8 group-DMAs on the SWDGE
    queue, issued BEFORE everything else; the mask DMA rides behind the
    fourth group.  (Previously 1.6MB of ddi loads sat in front of the X
    stream and delayed the first matmul by ~4.5us.)
  - A short burst of dummy matmuls at program start ramps the PE
    p-state off the 0.65/1.2GHz launch clocks while the first X group
    is still in flight.
"""

import sys

for _p in ("/opt/trn_rl_repo", "/root/.axon_site/_ro/trn_rl_repo"):
    if _p not in sys.path:
        sys.path.insert(0, _p)

import numpy as np
import ml_dtypes

B, D = 4096, 2048
NBLK = 128
NCOL = 512
NSLOT = 6
NIN = NCOL + NBLK * NSLOT  # 1280 cols per core
NCHUNK = 16  # 256-row DoubleRow chunks
NGRP = 8  # 2 chunks per DMA group
GBYTES = 2 * 2 * NIN  # 5120 per-partition bytes per group
N_WARM = 7

# (J, [row-block indices; -1 = dummy]) per core; tile (i,J) covers
# G[128i:128(i+1), 512J:512(J+1)], exists iff i <= 4J+3.
CORE_ASSIGN = [
    (3, [0, 1, 2, 3, 4, 5]),
    (3, [6, 7, 8, 9, 10, 11]),
    (3, [12, 13, 14, 15, -1, -1]),
    (2, [0, 1, 2, 3, 4, 5]),
    (2, [6, 7, 8, 9, 10, 11]),
    (1, [0, 1, 2, 3, 4, 5]),
    (1, [6, 7, -1, -1, -1, -1]),
    (0, [0, 1, 2, 3, -1, -1]),
]

_CACHE = {}


def _build():
    import concourse.mybir as mybir
    from concourse import bacc
    from concourse.tile import TileContext

    f32 = mybir.dt.float32
    bf16 = mybir.dt.bfloat16
    fp8 = mybir.dt.float8e5
    op = mybir.AluOpType

    nc = bacc.Bacc("TRN2", target_bir_lowering=False, debug=False, num_devices=8)

    # [128, 40960] fp8: partition p, chunk k, half i holds the 1280 xin
    # columns of batch row 256k + 128i + p at byte offset 2560k + 1280i.
    xin_d = nc.dram_tensor("xin", [128, NCHUNK * 2 * NIN], fp8, kind="ExternalInput")
    msk_d = nc.dram_tensor("msk", [128, NSLOT * NCOL], bf16, kind="ExternalInput")
    out_d = nc.dram_tensor("out", [128, NSLOT], f32, kind="ExternalOutput")

    with TileContext(nc) as tc:
        with (
            tc.tile_pool(name="const", bufs=1) as cpool,
            tc.tile_pool(name="xg", bufs=NGRP) as xpool,
            tc.tile_pool(name="psum", bufs=NSLOT, space="PSUM") as ppool,
            tc.tile_pool(name="scr", bufs=2) as spool,
        ):
            psums = [
                ppool.tile([128, NCOL], f32, tag="gps", name=f"gps{t}")
                for t in range(NSLOT)
            ]

            # --- PE p-state warmup on a zeroed tile, BEFORE the DMA issues
            # hog the gpsimd queue; result overwritten by the k=0
            # start=True matmul into the same PSUM bank ---
            warm = cpool.tile([128, 2, NCOL], fp8, tag="warm")
            nc.vector.memset(warm, 0)
            for _ in range(N_WARM):
                nc.tensor.matmul(
                    out=psums[0],
                    lhsT=warm[:, :, :NBLK],
                    rhs=warm,
                    start=True,
                    stop=True,
                    perf_mode=mybir.MatmulPerfMode.DoubleRow,
                )

            # --- X stream, masks behind group 3 ---
            grps = []
            msk = None
            for g in range(NGRP):
                xt = xpool.tile([128, 2, 2, NIN], fp8, tag="xg", name=f"xg{g}")
                nc.gpsimd.dma_start(
                    out=xt,
                    in_=xin_d.ap()[:, g * GBYTES : (g + 1) * GBYTES].rearrange(
                        "p (k i c) -> p k i c", k=2, i=2
                    ),
                )
                grps.append(xt)
                if g == 3:
                    msk = cpool.tile([128, NSLOT * NCOL], bf16, tag="msk")
                    nc.gpsimd.dma_start(out=msk, in_=msk_d.ap())

            # --- Gram matmuls: 6 x N=512 fp8 DR per 256-row chunk ---
            for k in range(NCHUNK):
                xt = grps[k // 2]
                kk = k % 2
                for t in range(NSLOT):
                    c0 = NCOL + t * NBLK
                    nc.tensor.matmul(
                        out=psums[t],
                        lhsT=xt[:, kk, :, c0 : c0 + NBLK],
                        rhs=xt[:, kk, :, 0:NCOL],
                        start=(k == 0),
                        stop=(k == NCHUNK - 1),
                        perf_mode=mybir.MatmulPerfMode.DoubleRow,
                    )

            # --- fused mask-multiply-reduce on DVE (Pool cannot read PSUM) ---
            out_sb = cpool.tile([128, NSLOT], f32, tag="out")
            for t in range(NSLOT):
                junk = spool.tile([128, NCOL], f32, tag="junk")
                nc.vector.scalar_tensor_tensor(
                    out=junk,
                    in0=psums[t],
                    scalar=1.0,
                    in1=msk[:, t * NCOL : (t + 1) * NCOL],
                    op0=op.mult,
                    op1=op.mult,
                    accum_out=out_sb[:, t : t + 1],
                )

            nc.gpsimd.dma_start(out=out_d.ap(), in_=out_sb)

    nc.compile()
    return nc


def _in_maps(drug_probs, ddi_matrix):
    xq = drug_probs.astype(ml_dtypes.float8_e5m2)
    bpos = ddi_matrix > 0
    U = np.triu(bpos | bpos.T, 1).astype(ml_dtypes.bfloat16)
    normalizer = max(float(np.count_nonzero(U)), 1.0)

    zero_x = np.zeros((B, NBLK), dtype=ml_dtypes.float8_e5m2)
    zero_m = np.zeros((NBLK, NCOL), dtype=ml_dtypes.bfloat16)
    maps = []
    for J, slots in CORE_ASSIGN:
        xin = np.concatenate(
            [xq[:, J * NCOL : (J + 1) * NCOL]]
            + [xq[:, i * NBLK : (i + 1) * NBLK] if i >= 0 else zero_x for i in slots],
            axis=1,
        )  # [4096, 1280]
        # pre-transpose: [128, 16*2*1280], 5120B contiguous per group line
        xin = np.ascontiguousarray(
            xin.reshape(NCHUNK, 2, 128, NIN)
            .transpose(2, 0, 1, 3)
            .reshape(128, NCHUNK * 2 * NIN)
        )
        msk = np.concatenate(
            [
                U[i * NBLK : (i + 1) * NBLK, J * NCOL : (J + 1) * NCOL]
                if i >= 0
                else zero_m
                for i in slots
            ],
            axis=1,
        )
        maps.append({"xin": xin, "msk": np.ascontiguousarray(msk)})
    return maps, normalizer


def kernel(drug_probs, ddi_matrix, **_run_kwargs):
    from concourse.bass_utils import run_bass_kernel_spmd

    if "nc" not in _CACHE:
        _CACHE["nc"] = _build()
    nc = _CACHE["nc"]

    maps, normalizer = _in_maps(np.asarray(drug_probs), np.asarray(ddi_matrix))
    res = run_bass_kernel_spmd(nc, maps, list(range(8)), **_run_kwargs)
    _CACHE["last_result"] = res

    gsum = 0.0
    for core_out in res.results:
        gsum += core_out["out"].astype(np.float64).sum()
    return np.asarray(gsum / (B * normalizer), dtype=np.float32)
